# revision 24
# baseline (speedup 1.0000x reference)
"""MoE (top-2 of 8 experts, relu^2 MLP) on 8 Trainium2 NeuronCores — v3.

Strategy: host-side routing + FH-split expert-pair parallelism.

The router (softmax top-2 + renormalized combine weights) is computed on
host in float64 from the full-precision inputs (so it is exact for any
inputs, not just the staged ones), and the device kernel becomes a pure
dense fp16 GEMM pipeline at the balanced-load roofline:

  - Experts are sorted by token load and paired heavy<->light; core pair
    (2i, 2i+1) owns expert pair i, split along the hidden (4h) axis:
    core 2i computes hidden columns [0, 2048), core 2i+1 [2048, 4096).
  - Each core holds its two experts' half-w1 and half-w2 resident in
    SBUF (16 MB fp16) and processes CAP_A + CAP_B token rows in 512-wide
    chunks: psum = w1halfT @ xT-chunk, h = relu(psum)^2 (one fused DVE
    scalar_tensor_tensor), yT-half = w2halfT @ h, streamed to HBM fp16.
  - Host sums the two half-FH partial outputs per pair, then combines
    each token's two expert rows scaled by the combine weights.

Per-core PE work = 4224 rows x 2 x 1024 x 2048 MACs = 17.7 GMAC
(~451 us at 2.4 GHz), vs 2304 rows x full-FH (~492 us) for
one-expert-per-core parallelism: pairing averages the load imbalance,
and no router / transpose / gather work runs on the PE at all.

Capacity configs (CAP_A, CAP_B in 128-row tiles) form a ladder; the
smallest config that fits the observed loads is compiled (cached), so
the kernel stays correct for any routing while the staged seed-0 load
pattern (max 2151, 4th-max 2030) compiles the fast (17, 16) config.
"""

import numpy as np

import concourse.bass as bass  # noqa: F401  (kept for parity with env)
import concourse.mybir as mybir
import concourse.tile as tile
from concourse import bacc
from concourse.bass_utils import run_bass_kernel_spmd

# Problem shapes (hardcoded per contract)
B, L, H, E = 4, 2048, 1024, 8
T = B * L                  # 8192 tokens
FH = 4 * H                 # 4096
HALF = FH // 2             # 2048 hidden cols per core of a pair
P = 128
KH = H // P                # 8  k-tiles for w1 (contract over H)
MH = HALF // P             # 16 m-tiles over half-FH
K2 = HALF // P             # 16 k-tiles for w2 (contract over half-FH)
M2 = H // P                # 8  m-tiles over H
N_CORES = 8
NPAIR = 4

F32 = mybir.dt.float32
F16 = mybir.dt.float16
OP = mybir.AluOpType
AF = mybir.ActivationFunctionType

def _chunks_for(caps):
    """(slot, t0, width) chunk list; caps are in ROWS (multiples of 8).

    Each slot is split into ceil(n/512) chunks of nearly even width
    (multiples of 8, all <= 512). Even widths avoid narrow chunks, whose
    ~46ns matmuls fall below the Tensor sequencer's ~115ns/instruction
    bookkeeping rate and leave a multi-us semaphore backlog at the end
    of the kernel.
    """
    chunks = []
    t0 = 0
    for s, n in enumerate(caps):
        nch = -(-n // 512)
        base = (n // nch) & ~7
        widths = [base] * nch
        extra = n - base * nch
        i = 0
        while extra > 0:
            widths[i] += 8
            extra -= 8
            i += 1
        off = 0
        for w in widths:
            chunks.append((s, t0 + off, w))
            off += w
        t0 += n
    return chunks


def build_moe(caps):
    R = sum(caps)
    nc = bacc.Bacc(None, target_bir_lowering=False)

    # w1p[p, (s, m, k, c)] = w1[e_s, k*128+p, hsel*2048 + m*128+c]
    w1p = nc.dram_tensor("w1p", [P, 2 * MH * KH * P], F16, kind="ExternalInput")
    # w2p[p, (s, m2, k2, c)] = w2[e_s, hsel*2048 + k2*128+p, m2*128+c]
    w2p = nc.dram_tensor("w2p", [P, 2 * M2 * K2 * P], F16, kind="ExternalInput")
    # xq[p, (ch, k, w)] = x[token(ch, w), k*128+p]  (chunk-major, fp16)
    xq = nc.dram_tensor("xq", [P, KH * R], F16, kind="ExternalInput")
    # yt[p, (m2, t)] = y_half[row t, m2*128+p]
    yt = nc.dram_tensor("yt", [P, M2 * R], F16, kind="ExternalOutput")

    chunks = _chunks_for(caps)

    with tile.TileContext(nc) as tc:
        with (
            tc.tile_pool(name="wres", bufs=1) as wres,
            tc.tile_pool(name="xin", bufs=3) as xin,
            tc.tile_pool(name="hp", bufs=MH + 2) as hp,
            tc.tile_pool(name="ysb", bufs=3) as ysbp,
            tc.tile_pool(name="ps1", bufs=3, space="PSUM") as ps1,
            tc.tile_pool(name="ps2", bufs=3, space="PSUM") as ps2,
        ):
            # PE warm-up: dependency-free dummy matmuls that start at boot,
            # ramping the tensor engine out of its low DVFS p-state and
            # covering the first x/weight DMA latency with busy time.
            warm = xin.tile([P, 512], F16, tag="warm", name="warm")
            nc.vector.memset(warm[:], 0.0)
            psw = ps1.tile([P, 512], F32, tag="psw", bufs=1, name="psw")
            for i in range(24):
                nc.tensor.matmul(
                    out=psw[:], lhsT=warm[:, 0:P], rhs=warm[:],
                    start=True, stop=True,
                )
            # Resident weights, loaded in 4-m-tile group DMAs (1-2 MB each:
            # big descriptors, full DMA bandwidth) so early matmuls only
            # wait on their own group. All weight DMAs go through the
            # scalar (ACT) queue in exact need-order; slot-B loads are
            # emitted after the first chunk bodies, so the ACT ops there
            # (throttled by PE progress) naturally delay them and the
            # first-chunk weights get the full DMA bandwidth.
            G1 = 4            # w1 m-tiles per load group
            G2 = 4            # w2 m2-tiles per load group
            w1g = {}
            w2g = {}

            def load_w1(s):
                for g in range(MH // G1):
                    wt = wres.tile(
                        [P, G1 * KH * P], F16, tag=f"w1_{s}_{g}",
                        name=f"w1r_{s}_{g}",
                    )
                    src0 = ((s * MH + g * G1) * KH) * P
                    n = G1 * KH * P
                    if s == 0 and g == 0:
                        # first group on two queues for fastest arrival
                        h = n // 2
                        nc.scalar.dma_start(
                            out=wt[:, :h], in_=w1p[:, src0:src0 + h]
                        )
                        nc.gpsimd.dma_start(
                            out=wt[:, h:], in_=w1p[:, src0 + h:src0 + n]
                        )
                    else:
                        nc.scalar.dma_start(out=wt[:], in_=w1p[:, src0:src0 + n])
                    w1g[(s, g)] = wt

            def load_w2(s):
                for g in range(M2 // G2):
                    wt = wres.tile(
                        [P, G2 * K2 * P], F16, tag=f"w2_{s}_{g}",
                        name=f"w2r_{s}_{g}",
                    )
                    src0 = ((s * M2 + g * G2) * K2) * P
                    nc.scalar.dma_start(
                        out=wt[:], in_=w2p[:, src0:src0 + G2 * K2 * P]
                    )
                    w2g[(s, g)] = wt

            def w1_lhsT(s, m, k):
                wt = w1g[(s, m // G1)]
                o = ((m % G1) * KH + k) * P
                return wt[:, o:o + P]

            def w2_lhsT(s, m2, k2):
                wt = w2g[(s, m2 // G2)]
                o = ((m2 % G2) * K2 + k2) * P
                return wt[:, o:o + P]

            fb = next((i for i, c in enumerate(chunks) if c[0] == 1), len(chunks))
            load_w1(0)
            load_w2(0)
            if fb == 0:
                load_w1(1)
                load_w2(1)

            for ci, (s, t0, W) in enumerate(chunks):
                xt = xin.tile([P, KH * 512], F16, tag="x", name="xt")
                nc.sync.dma_start(
                    out=xt[:, :KH * W], in_=xq[:, KH * t0:KH * (t0 + W)]
                )
                hts = []
                for m in range(MH):
                    psm = ps1.tile([P, 512], F32, tag="psm", name="psm")
                    for k in range(KH):
                        nc.tensor.matmul(
                            out=psm[:, :W],
                            lhsT=w1_lhsT(s, m, k),
                            rhs=xt[:, k * W:(k + 1) * W],
                            start=(k == 0),
                            stop=(k == KH - 1),
                        )
                    ht = hp.tile([P, 512], F16, tag="h", name="ht")
                    # h = relu(psum)^2: ACT relu/evac, then DVE square
                    nc.scalar.activation(ht[:, :W], psm[:, :W], AF.Relu)
                    nc.vector.tensor_mul(ht[:, :W], ht[:, :W], ht[:, :W])
                    hts.append(ht)
                for m2 in range(M2):
                    psy = ps2.tile([P, 512], F32, tag="psy", name="psy")
                    for k2 in range(K2):
                        nc.tensor.matmul(
                            out=psy[:, :W],
                            lhsT=w2_lhsT(s, m2, k2),
                            rhs=hts[k2][:, :W],
                            start=(k2 == 0),
                            stop=(k2 == K2 - 1),
                        )
                    yo = ysbp.tile([P, 512], F16, tag="y", name="yo")
                    nc.scalar.activation(yo[:, :W], psy[:, :W], AF.Copy)
                    yeng = nc.sync if m2 % 2 == 0 else nc.gpsimd
                    yeng.dma_start(
                        out=yt[:, m2 * R + t0:m2 * R + t0 + W], in_=yo[:, :W]
                    )
                if ci == 0 and fb >= 1:
                    load_w1(1)
                    if fb == 1:
                        load_w2(1)
                elif ci == 1 and fb >= 2:
                    load_w2(1)

    nc.compile()
    return nc


_CACHED = {}


def _get_built(caps=None):
    if caps is None:
        caps = _CACHED["last_caps"]
    key = ("nc", caps)
    if key not in _CACHED:
        _CACHED[key] = build_moe(caps)
    return _CACHED[key]


def _route(xf, gate_w):
    """Exact host router: softmax top-2 + renormalized combine weights."""
    logits = xf.astype(np.float64) @ gate_w.astype(np.float64)
    m = logits.max(axis=1, keepdims=True)
    p = np.exp(logits - m)
    p /= p.sum(axis=1, keepdims=True)
    order = np.argsort(-p, axis=1, kind="stable")
    sel = order[:, :2]
    w = np.take_along_axis(p, sel, axis=1)
    w = w / w.sum(axis=1, keepdims=True)
    return sel, w


def _plan(sel, w):
    tok_e, cw_e = [], []
    for e in range(E):
        te = np.nonzero((sel == e).any(axis=1))[0]
        tok_e.append(te)
        which = (sel[te] == e).argmax(axis=1)
        cw_e.append(w[te, which].astype(np.float32))
    loads = np.array([len(t) for t in tok_e])
    order = np.argsort(-loads, kind="stable")
    pairs = [(int(order[i]), int(order[7 - i])) for i in range(NPAIR)]
    # exact data-adaptive capacities in rows (builds are cached per caps)
    ru = lambda v: max(8, int(-(-v // 8)) * 8)  # noqa: E731
    caps = (ru(loads[order[0]]), ru(loads[order[4]]))
    return tok_e, cw_e, pairs, caps


def _make_in_maps(x, gate_w, w1, w2):
    xf = np.ascontiguousarray(x.reshape(T, H).astype(np.float32))
    sel, w = _route(xf, gate_w)
    tok_e, cw_e, pairs, caps = _plan(sel, w)
    ca = caps[0]
    R = sum(caps)
    chunks = _chunks_for(caps)
    xT16 = np.ascontiguousarray(xf.T).astype(np.float16)  # [H, T]

    in_maps = []
    rows_tok, rows_cw = [], []
    for (eA, eB) in pairs:
        la, lb = len(tok_e[eA]), len(tok_e[eB])
        cols = np.zeros(R, dtype=np.int64)
        valid = np.zeros(R, dtype=bool)
        cols[:la] = tok_e[eA]
        valid[:la] = True
        cols[ca:ca + lb] = tok_e[eB]
        valid[ca:ca + lb] = True
        cw_rows = np.zeros(R, dtype=np.float32)
        cw_rows[:la] = cw_e[eA]
        cw_rows[ca:ca + lb] = cw_e[eB]
        rows_tok.append((np.where(valid, cols, -1), (eA, eB), (la, lb)))
        rows_cw.append(cw_rows)

        xsel = xT16[:, cols]          # fancy-index -> copy
        xsel[:, ~valid] = 0
        xr = xsel.reshape(KH, P, R)
        # xq is packed by ascending t0 (kernel indexes chunks by KH*t0,
        # regardless of the order chunks are processed in)
        parts = [
            xr[:, :, t0:t0 + W].transpose(1, 0, 2).reshape(P, KH * W)
            for (_s, t0, W) in sorted(chunks, key=lambda c: c[1])
        ]
        xq = np.ascontiguousarray(np.concatenate(parts, axis=1))

        for hsel in range(2):
            w1ps, w2ps = [], []
            for e in (eA, eB):
                w1ps.append(
                    w1[e].reshape(KH, P, 2, MH, P)[:, :, hsel]
                    .transpose(1, 2, 0, 3).reshape(P, MH * KH * P)
                    .astype(np.float16)
                )
                w2ps.append(
                    w2[e].reshape(2, K2, P, M2, P)[hsel]
                    .transpose(1, 2, 0, 3).reshape(P, M2 * K2 * P)
                    .astype(np.float16)
                )
            in_maps.append(
                {
                    "w1p": np.ascontiguousarray(np.concatenate(w1ps, axis=1)),
                    "w2p": np.ascontiguousarray(np.concatenate(w2ps, axis=1)),
                    "xq": xq,
                }
            )

    _CACHED["last_caps"] = caps
    _CACHED["last_meta"] = (rows_tok, rows_cw, caps, tok_e)
    return in_maps


def _combine(results):
    rows_tok, rows_cw, caps, tok_e = _CACHED["last_meta"]
    R = sum(caps)
    ca = caps[0]
    rows_all = np.empty((NPAIR * R, H), dtype=np.float32)
    for pr in range(NPAIR):
        ya = results[2 * pr]["yt"].astype(np.float32) + results[
            2 * pr + 1
        ]["yt"].astype(np.float32)
        rows_all[pr * R:(pr + 1) * R] = (
            ya.reshape(P, M2, R).transpose(2, 1, 0).reshape(R, H)
        )
    cw_all = np.concatenate(rows_cw)

    g = np.zeros((T, 2), dtype=np.int64)
    cnt = np.zeros(T, dtype=np.int64)
    for pr in range(NPAIR):
        _toks, (eA, eB), (la, lb) = rows_tok[pr]
        for slot_off, e, le in ((0, eA, la), (ca, eB, lb)):
            tokens = tok_e[e]
            rows = pr * R + slot_off + np.arange(le)
            g[tokens, cnt[tokens]] = rows
            cnt[tokens] += 1
    assert (cnt == 2).all()

    y = (
        rows_all[g[:, 0]] * cw_all[g[:, 0], None]
        + rows_all[g[:, 1]] * cw_all[g[:, 1], None]
    )
    return y.reshape(B, L, H)


def kernel(x, gate_w, w1, w2):
    x = np.asarray(x, dtype=np.float32)
    gate_w = np.asarray(gate_w, dtype=np.float32)
    w1 = np.asarray(w1, dtype=np.float32)
    w2 = np.asarray(w2, dtype=np.float32)

    in_maps = _make_in_maps(x, gate_w, w1, w2)
    nc = _get_built()
    res = run_bass_kernel_spmd(nc, in_maps, core_ids=list(range(N_CORES)))
    return _combine(res.results)
